# revision 6
# baseline (speedup 1.0000x reference)
"""Trainium2 Bass kernel for the capsule-routing layer (nn_Caps_Layer).

Computation (per batch b of x [B, S, D], W [D, 25]):
  u_hat = (x_b @ W).reshape(S, 5, 5)           # [S, n, k], col = n*5+k
  b0 = 0;  for 4 routing iters:
    c = softmax_n(b)                            # over the 5 capsules
    v[n,k] = sum_s c[n,s] u_hat[s,n,k]
    out = v / sqrt(sum_k v^2 + 1e-7)
    b[n,s] = sum_k out[n,k] u_hat[s,n,k]
Returns out [B, 5, 5].

Sharding: pure data-parallel over batch across 8 NeuronCores (16 batches
each); W replicated; no collectives.

v3 pipeline (per core):
  - x loaded f32 at full HWDGE rate (sync engine); cast f32->fp16 split
    between ACT (first half) and Pool (second half) - both otherwise idle.
  - PE transposes at fp16 (1 cyc/row) into fp16 PSUM; psum->sbuf copies
    on DVE (fp16 2x mode) and ACT.
  - main matmul fp16: W.T @ xT accumulated into a [128,512] f32 PSUM tile
    stacked 4 batches deep (partition offsets 0/32/64/96 via explicit
    tile_position) so the uhT copy and back-transposes amortize 4x.
  - routing: fp16 elementwise, partition sums via fp16 ones-matmul; the
    squash norm is folded into the broadcast (outv = v * cs*rnrm is
    broadcast, so no separate prn matmul / blog multiply); rsqrt is
    exp(-0.5*ln(cs^2*s2+eps)) so ACT stays on ONE table set (copy/exp/ln/
    square all in natural_log_exp_and_others -> no table thrash).
  - groups (8,4,4) with per-group-parity tile tags so consecutive groups'
    routing chains overlap instead of serializing through shared tiles.
"""

from contextlib import ExitStack

import math

import numpy as np

import concourse.bass as bass
import concourse.tile as tile
from concourse import mybir, masks

F32 = mybir.dt.float32
FP16 = mybir.dt.float16
BF16 = mybir.dt.bfloat16
AX = mybir.AxisListType
OP = mybir.AluOpType
AF = mybir.ActivationFunctionType

N_CORES = 8
B_FULL, S, D = 128, 512, 768
NCAP, KDIM = 5, 5
NK = NCAP * KDIM  # 25
ROUTINGS = 4
T_EPS = 1e-7

ND = D // 128   # 6 d-blocks
NSB = S // 128  # 4 s-blocks (= s_hi)
SI = 4          # batches stacked per [128,512] psum tile (offsets 0/32/64/96)


def emit(ctx, tc, out, x, w, b_loc=16, groups=(8, 4, 4)):
    """Emit the per-core kernel IR.

    out: [1, b_loc*25] f32; x: [b_loc*512, 768] f32; w: [768, 25] f32.
    """
    nc = tc.nc
    groups = list(groups)
    assert sum(groups) == b_loc
    assert all(g % SI == 0 for g in groups)

    const_pool = ctx.enter_context(tc.tile_pool(name="const", bufs=1))
    xraw_pool = ctx.enter_context(tc.tile_pool(name="xraw", bufs=3))
    xnat_pool = ctx.enter_context(tc.tile_pool(name="xnat", bufs=3))
    ptr_pool = ctx.enter_context(tc.tile_pool(name="ptr", bufs=2, space="PSUM"))
    xt_pool = ctx.enter_context(tc.tile_pool(name="xt", bufs=3))
    pu_pool = ctx.enter_context(tc.tile_pool(name="pu", bufs=2, space="PSUM"))
    uhT_pool = ctx.enter_context(tc.tile_pool(name="uhT", bufs=2))
    uh_pool = ctx.enter_context(tc.tile_pool(name="uh", bufs=2))
    rt_pool = ctx.enter_context(tc.tile_pool(name="rt", bufs=2))
    pv_pool = ctx.enter_context(tc.tile_pool(name="pv", bufs=1, space="PSUM"))

    # --- constants ---
    ident = const_pool.tile([128, 128], F32)
    masks.make_identity(nc, ident[:])
    ident_h = const_pool.tile([128, 128], FP16)
    nc.scalar.copy(ident_h[:], ident[:])
    w_raw = const_pool.tile([128, ND * NK], F32)
    # DRAM [768, 25] -> [128, (dblk, nk)]
    nc.sync.dma_start(
        w_raw[:].rearrange("p (nb k) -> p nb k", nb=ND),
        w.rearrange("(nb p) k -> p nb k", p=128),
    )
    w_sb = const_pool.tile([128, ND * NK], FP16)
    nc.scalar.copy(w_sb[:], w_raw[:])

    ones_col_h = const_pool.tile([128, 1], FP16)
    nc.gpsimd.memset(ones_col_h[:], 1.0)
    ones_row = const_pool.tile([1, 128], F32)
    nc.gpsimd.memset(ones_row[:], 1.0)
    eps1 = const_pool.tile([1, 1], F32)
    nc.gpsimd.memset(eps1[:], T_EPS)
    # bias for folding cs=1/5 into the iter-0 rsqrt: exp(-.5*ln(..)+ln(cs))
    lncs = const_pool.tile([1, 1], F32)
    nc.gpsimd.memset(lncs[:], math.log(1.0 / NCAP))

    # HAM warm-up overlapping the first DMA (real matmuls at 2.4GHz after
    # ~3us of continuous PE activity; later the 6 pu-matmuls per batch and
    # the routing pv matmuls keep the clock gate awake).
    wps = pv_pool.tile([1, 128], F32, tag="pv_0")
    for _ in range(24):
        nc.tensor.matmul(wps[:], ones_col_h[:], ident_h[:], start=True, stop=True)

    CSPLIT = NSB * D // 2  # fp16-cast split point: ACT takes [0:half], Pool rest

    b_off = 0
    for g, G in enumerate(groups):
        gp = g % 2  # tag parity: lets consecutive groups' routing overlap
        GSI = G // SI
        uh = uh_pool.tile([128, G * NSB * NK], FP16, tag="uh")
        # free layout: (st, sh, si, n, k)
        uh_ap = uh[:].rearrange(
            "p (st sh si n k) -> p st sh si n k", st=GSI, sh=NSB, si=SI, n=NCAP
        )
        pu = None
        for bi in range(G):
            b = b_off + bi
            st, si = bi // SI, bi % SI
            if si == 0:
                pu = pu_pool.tile([128, S], F32, tag="pu")
            # --- load x rows for batch b: [512, 768] -> f32 [128,(sb,d)] ---
            x_raw = xraw_pool.tile([128, NSB * D], F32)
            nc.sync.dma_start(
                x_raw[:].rearrange("p (sb d) -> p sb d", sb=NSB),
                x[b * S:(b + 1) * S, :].rearrange("(sb p) d -> p sb d", p=128),
            )
            # --- cast to fp16 on ACT (first half) + Pool (second half) ---
            x_nat = xnat_pool.tile([128, NSB * D], FP16)
            nc.scalar.copy(x_nat[:, 0:CSPLIT], x_raw[:, 0:CSPLIT])
            nc.gpsimd.tensor_copy(x_nat[:, CSPLIT:], x_raw[:, CSPLIT:])
            # --- transpose to xT [128 d_lo, (dblk, s)] ---
            xT = xt_pool.tile([128, ND * S], FP16)
            xT3 = xT[:].rearrange("p (db s) -> p db s", db=ND)
            for db in range(ND):
                ptr = ptr_pool.tile([128, S], FP16, tag="ptr")
                for sb_i in range(NSB):
                    nc.tensor.transpose(
                        ptr[:, sb_i * 128:(sb_i + 1) * 128],
                        x_nat[:, sb_i * D + db * 128:sb_i * D + (db + 1) * 128],
                        ident_h[:],
                    )
                # psum -> sbuf copy; 2 on ACT, 4 on DVE (fp16 2x mode)
                if db in (1, 4):
                    nc.scalar.copy(xT3[:, db], ptr[:])
                else:
                    nc.vector.tensor_copy(xT3[:, db], ptr[:])
                nc.tensor.matmul(
                    pu[si * 32:si * 32 + NK, :],
                    w_sb[:, db * NK:(db + 1) * NK],
                    xT3[:, db],
                    start=(db == 0),
                    stop=(db == ND - 1),
                    tile_position=(0, si * 32),
                )
            if si == SI - 1:
                # --- stack of 4 batches complete: back to natural layout ---
                uhT = uhT_pool.tile([128, S], FP16, tag="uhT")
                nc.vector.tensor_copy(uhT[:], pu[:])
                pnat = ptr_pool.tile([128, NSB * 128], FP16, tag="ptr")
                for sh in range(NSB):
                    nc.tensor.transpose(
                        pnat[:, sh * 128:(sh + 1) * 128],
                        uhT[:, sh * 128:(sh + 1) * 128],
                        ident_h[:],
                    )
                # slice away the 32-alignment padding: (sh, si, 32) -> (sh, si, 25)
                nc.vector.tensor_copy(
                    uh_ap[:, st],
                    pnat[:]
                    .rearrange("p (sh si c) -> p sh si c", sh=NSB, si=SI)[:, :, :, 0:NK]
                    .rearrange("p sh si (n k) -> p sh si n k", n=NCAP),
                )

        # ---------------- routing for this group ----------------
        # Per iter i (cs = 1/5 on iter 0 via the lncs bias, else 1):
        #   c    = softmax_n(b)                   (skipped on i=0: c = 1/5)
        #   v    = sum_s c*u_hat                  (PE partition-sum, raw v)
        #   rnrm = cs * (cs^2*|v|^2 + eps)^-1/2   (ACT: exp(-.5*ln(.)+ln cs))
        #   outv = v * rnrm                       (= squashed output)
        #   b'   = sum_k outv*u_hat               (pvb broadcast + reduce)
        blog = None
        for it in range(ROUTINGS):
            cs = 1.0 / NCAP if it == 0 else 1.0
            if it == 0:
                t_ap = uh_ap
            else:
                expb = rt_pool.tile([128, G * NSB * NCAP], BF16, tag=f"expb{gp}")
                nc.scalar.activation(expb[:], blog[:], AF.Exp)
                den = rt_pool.tile([128, G * NSB], F32, tag=f"den{gp}")
                nc.vector.reduce_sum(
                    den[:],
                    expb[:].rearrange("p (bs n) -> p bs n", n=NCAP),
                    axis=AX.X,
                )
                rden = rt_pool.tile([128, G * NSB], F32, tag=f"rden{gp}")
                nc.vector.reciprocal(rden[:], den[:])
                c = rt_pool.tile([128, G * NSB * NCAP], FP16, tag=f"c{gp}")
                nc.vector.tensor_tensor(
                    c[:].rearrange(
                        "p (st sh si n) -> p st sh si n", st=GSI, sh=NSB, si=SI
                    ),
                    expb[:].rearrange(
                        "p (st sh si n) -> p st sh si n", st=GSI, sh=NSB, si=SI
                    ),
                    rden[:]
                    .rearrange("p (st sh si) -> p st sh si", st=GSI, sh=NSB)
                    .unsqueeze(4)
                    .broadcast_to((128, GSI, NSB, SI, NCAP)),
                    op=OP.mult,
                )
                c_b = (
                    c[:]
                    .rearrange(
                        "p (st sh si n) -> p st sh si n", st=GSI, sh=NSB, si=SI
                    )
                    .unsqueeze(5)
                    .broadcast_to((128, GSI, NSB, SI, NCAP, KDIM))
                )
                t = rt_pool.tile([128, G * NSB * NK], FP16, tag=f"t{gp}")
                t_ap = t[:].rearrange(
                    "p (st sh si n k) -> p st sh si n k",
                    st=GSI, sh=NSB, si=SI, n=NCAP,
                )
                nc.vector.tensor_tensor(t_ap, uh_ap, c_b, op=OP.mult)
            # ---- v[b,n,k] = sum_s t: partition sum via ones matmul ----
            pv = pv_pool.tile([1, G * NK], F32, tag=f"pv_{gp}")
            for sh in range(NSB):
                nc.tensor.matmul(
                    pv[:],
                    ones_col_h[:],
                    t_ap[:, :, sh],
                    start=(sh == 0),
                    stop=(sh == NSB - 1),
                )
            # ---- norm branch (all tiny): v_sb copy then Square, both on ACT
            v_sb = rt_pool.tile([1, G * NK], F32, tag=f"v_sb{gp}")
            nc.scalar.copy(v_sb[:], pv[:])
            sq = rt_pool.tile([1, G * NK], F32, tag=f"sq{gp}")
            nc.scalar.activation(sq[:], v_sb[:], AF.Square)
            s2 = rt_pool.tile([1, G * NCAP], F32, tag=f"s2{gp}")
            nc.vector.reduce_sum(
                s2[:], sq[:].rearrange("p (bn k) -> p bn k", k=KDIM), axis=AX.X
            )
            lns = rt_pool.tile([1, G * NCAP], F32, tag=f"lns{gp}")
            nc.scalar.activation(
                lns[:], s2[:], AF.Ln, bias=eps1[:], scale=cs * cs
            )
            rnrm = rt_pool.tile([1, G * NCAP], F32, tag=f"rnrm{gp}")
            if it == 0:
                nc.scalar.activation(rnrm[:], lns[:], AF.Exp, bias=lncs[:], scale=-0.5)
            else:
                nc.scalar.activation(rnrm[:], lns[:], AF.Exp, scale=-0.5)
            # outv = v * cs*rnrm  (squashed output; b-update = sum_k outv*uh)
            outv = rt_pool.tile([1, G * NK], F32, tag=f"outv{gp}")
            nc.vector.tensor_tensor(
                outv[:].rearrange("p (bn k) -> p bn k", k=KDIM),
                v_sb[:].rearrange("p (bn k) -> p bn k", k=KDIM),
                rnrm[:].unsqueeze(2).broadcast_to((1, G * NCAP, KDIM)),
                op=OP.mult,
            )
            if it < ROUTINGS - 1:
                pvb = pv_pool.tile([128, G * NK], F32, tag=f"pvb{gp}")
                nc.tensor.matmul(pvb[:], ones_row[:], outv[:], start=True, stop=True)
                tmp = rt_pool.tile([128, G * NSB * NK], FP16, tag=f"tmp{gp}")
                tmp_ap = tmp[:].rearrange(
                    "p (st sh si n k) -> p st sh si n k",
                    st=GSI, sh=NSB, si=SI, n=NCAP,
                )
                nc.vector.tensor_tensor(
                    tmp_ap,
                    uh_ap,
                    pvb[:]
                    .rearrange("p (st si n k) -> p st si n k", st=GSI, si=SI, n=NCAP)
                    .unsqueeze(2)
                    .broadcast_to((128, GSI, NSB, SI, NCAP, KDIM)),
                    op=OP.mult,
                )
                blog = rt_pool.tile([128, G * NSB * NCAP], FP16, tag=f"blog{gp}")
                with nc.allow_low_precision("5-term k-reduce of bounded logits"):
                    nc.vector.reduce_sum(
                        blog[:],
                        tmp[:].rearrange("p (bsn k) -> p bsn k", k=KDIM),
                        axis=AX.X,
                    )
            else:
                nc.sync.dma_start(
                    out[0:1, b_off * NK:(b_off + G) * NK],
                    outv[0:1, :],
                )
        b_off += G


def legalize_waits(nc):
    """This toolchain's walrus codegen accepts at most ONE sync wait per
    instruction ("Too many sync wait commands" otherwise) — and PE Matmult
    appears to take none safely. Hoist excess waits onto wait-only
    EventSemaphore instructions inserted just before, on the same engine
    (same pattern walrus already accepts for Tile's engine barriers)."""
    n = 0
    for fn in nc.m.functions:
        for blk in fn.blocks:
            new = []
            for inst in blk.instructions:
                si = inst.sync_info
                if si is not None and len(si.on_wait) > 0:
                    waits = list(si.on_wait)
                    keep = 0 if type(inst).__name__ == "InstMatmult" else 1
                    if len(waits) > keep:
                        for wt in waits[: len(waits) - keep]:
                            ev = mybir.InstEventSemaphore(
                                name=f"I-waitfix-{nc.next_id()}"
                            )
                            ev.engine = inst.engine
                            ev.sync_info = mybir.SyncInfo(on_wait=[wt], on_update=[])
                            new.append(ev)
                            n += 1
                        si.on_wait = waits[len(waits) - keep:]
                new.append(inst)
            blk.instructions = new
    return n


def build_caps_kernel(b_loc=16, groups=(8, 4, 4)):
    nc = bass.Bass(trn_type="TRN2", debug=False, target_bir_lowering=False)
    x = nc.dram_tensor("x", [b_loc * S, D], F32, kind="ExternalInput").ap()
    w = nc.dram_tensor("w", [D, NK], F32, kind="ExternalInput").ap()
    out = nc.dram_tensor("out", [1, b_loc * NK], F32, kind="ExternalOutput").ap()
    with tile.TileContext(nc) as tc:
        with ExitStack() as ctx:
            emit(ctx, tc, out, x, w, b_loc=b_loc, groups=groups)
    legalize_waits(nc)
    return nc


_KERNEL_CFG = dict(groups=(8, 4, 4))


def kernel(x: np.ndarray, W: np.ndarray) -> np.ndarray:
    from concourse.bass_utils import run_bass_kernel_spmd

    B, S_, D_ = x.shape
    assert (B, S_, D_) == (B_FULL, S, D)
    b_loc = B // N_CORES
    nc = build_caps_kernel(b_loc=b_loc, **_KERNEL_CFG)
    in_maps = [
        {
            "x": np.ascontiguousarray(
                x[i * b_loc:(i + 1) * b_loc].reshape(b_loc * S, D)
            ),
            "w": np.ascontiguousarray(W),
        }
        for i in range(N_CORES)
    ]
    res = run_bass_kernel_spmd(nc, in_maps, core_ids=list(range(N_CORES)))
    outs = [res.results[i]["out"].reshape(b_loc, NCAP, KDIM) for i in range(N_CORES)]
    return np.concatenate(outs, axis=0).astype(np.float32)


# revision 8
# speedup vs baseline: 1.3176x; 1.3176x over previous
"""Trainium2 Bass kernel for the capsule-routing layer (nn_Caps_Layer).

Computation (per batch b of x [B, S, D], W [D, 25]):
  u_hat = (x_b @ W).reshape(S, 5, 5)           # [S, n, k], col = n*5+k
  b0 = 0;  for 4 routing iters:
    c = softmax_n(b)                            # over the 5 capsules
    v[n,k] = sum_s c[n,s] u_hat[s,n,k]
    out = v / sqrt(sum_k v^2 + 1e-7)
    b[n,s] = sum_k out[n,k] u_hat[s,n,k]
Returns out [B, 5, 5].

Sharding: pure data-parallel over batch across 8 NeuronCores (16 batches
each); W replicated; no collectives.

v3 pipeline (per core):
  - x loaded f32 at full HWDGE rate (sync engine); cast f32->fp16 split
    between ACT (first half) and Pool (second half) - both otherwise idle.
  - PE transposes at fp16 (1 cyc/row) into fp16 PSUM; psum->sbuf copies
    on DVE (fp16 2x mode) and ACT.
  - main matmul fp16: W.T @ xT accumulated into a [128,512] f32 PSUM tile
    stacked 4 batches deep (partition offsets 0/32/64/96 via explicit
    tile_position) so the uhT copy and back-transposes amortize 4x.
  - routing: fp16 elementwise, partition sums via fp16 ones-matmul; the
    squash norm is folded into the broadcast (outv = v * cs*rnrm is
    broadcast, so no separate prn matmul / blog multiply); rsqrt is
    exp(-0.5*ln(cs^2*s2+eps)) so ACT stays on ONE table set (copy/exp/ln/
    square all in natural_log_exp_and_others -> no table thrash).
  - groups (8,4,4) with per-group-parity tile tags so consecutive groups'
    routing chains overlap instead of serializing through shared tiles.
"""

from contextlib import ExitStack

import math

import numpy as np

import concourse.bass as bass
import concourse.tile as tile
from concourse import mybir, masks

F32 = mybir.dt.float32
FP16 = mybir.dt.float16
BF16 = mybir.dt.bfloat16
AX = mybir.AxisListType
OP = mybir.AluOpType
AF = mybir.ActivationFunctionType

N_CORES = 8
B_FULL, S, D = 128, 512, 768
NCAP, KDIM = 5, 5
NK = NCAP * KDIM  # 25
ROUTINGS = 4
T_EPS = 1e-7

ND = D // 128   # 6 d-blocks
NSB = S // 128  # 4 s-blocks (= s_hi)
SI = 4          # batches stacked per [128,512] psum tile (offsets 0/32/64/96)


def emit(ctx, tc, out, x, w, b_loc=16, groups=(8, 4, 4)):
    """Emit the per-core kernel IR.

    out: [1, b_loc*25] f32; x: [b_loc*512, 768] f32; w: [768, 25] f32.
    """
    nc = tc.nc
    groups = list(groups)
    assert sum(groups) == b_loc
    assert all(g % SI == 0 for g in groups)

    const_pool = ctx.enter_context(tc.tile_pool(name="const", bufs=1))
    xraw_pool = ctx.enter_context(tc.tile_pool(name="xraw", bufs=3))
    xnat_pool = ctx.enter_context(tc.tile_pool(name="xnat", bufs=3))
    ptr_pool = ctx.enter_context(tc.tile_pool(name="ptr", bufs=2, space="PSUM"))
    xt_pool = ctx.enter_context(tc.tile_pool(name="xt", bufs=3))
    pu_pool = ctx.enter_context(tc.tile_pool(name="pu", bufs=2, space="PSUM"))
    uhT_pool = ctx.enter_context(tc.tile_pool(name="uhT", bufs=2))
    uh_pool = ctx.enter_context(tc.tile_pool(name="uh", bufs=2))
    rt_pool = ctx.enter_context(tc.tile_pool(name="rt", bufs=2))
    pv_pool = ctx.enter_context(tc.tile_pool(name="pv", bufs=1, space="PSUM"))

    # --- constants ---
    ident = const_pool.tile([128, 128], F32)
    masks.make_identity(nc, ident[:])
    ident_h = const_pool.tile([128, 128], FP16)
    nc.scalar.copy(ident_h[:], ident[:])
    w_raw = const_pool.tile([128, ND * NK], F32)
    # DRAM [768, 25] -> [128, (dblk, nk)]
    nc.sync.dma_start(
        w_raw[:].rearrange("p (nb k) -> p nb k", nb=ND),
        w.rearrange("(nb p) k -> p nb k", p=128),
    )
    w_sb = const_pool.tile([128, ND * NK], FP16)
    nc.scalar.copy(w_sb[:], w_raw[:])

    ones_col_h = const_pool.tile([128, 1], FP16)
    nc.gpsimd.memset(ones_col_h[:], 1.0)
    ones_row = const_pool.tile([1, 128], F32)
    nc.gpsimd.memset(ones_row[:], 1.0)
    eps1 = const_pool.tile([1, 1], F32)
    nc.gpsimd.memset(eps1[:], T_EPS)
    # bias for folding cs=1/5 into the iter-0 rsqrt: exp(-.5*ln(..)+ln(cs))
    lncs = const_pool.tile([1, 1], F32)
    nc.gpsimd.memset(lncs[:], math.log(1.0 / NCAP))

    # HAM warm-up overlapping the first DMA (real matmuls at 2.4GHz after
    # ~3us of continuous PE activity; later the 6 pu-matmuls per batch and
    # the routing pv matmuls keep the clock gate awake).
    wps = pv_pool.tile([1, 128], F32, tag="pv_0")
    for _ in range(24):
        nc.tensor.matmul(wps[:], ones_col_h[:], ident_h[:], start=True, stop=True)

    CSPLIT = NSB * D // 2  # fp16-cast split point: ACT takes [0:half], Pool rest

    b_off = 0
    for g, G in enumerate(groups):
        gp = g % 2  # tag parity: lets consecutive groups' routing overlap
        GSI = G // SI
        uh = uh_pool.tile([128, G * NSB * NK], FP16, tag="uh")
        # free layout: (st, sh, si, n, k)
        uh_ap = uh[:].rearrange(
            "p (st sh si n k) -> p st sh si n k", st=GSI, sh=NSB, si=SI, n=NCAP
        )
        pu = None
        for bi in range(G):
            b = b_off + bi
            st, si = bi // SI, bi % SI
            if si == 0:
                pu = pu_pool.tile([128, S], F32, tag="pu")
            # --- load x rows for batch b: [512, 768] -> f32 [128,(sb,d)] ---
            x_raw = xraw_pool.tile([128, NSB * D], F32)
            nc.sync.dma_start(
                x_raw[:].rearrange("p (sb d) -> p sb d", sb=NSB),
                x[b * S:(b + 1) * S, :].rearrange("(sb p) d -> p sb d", p=128),
            )
            # --- cast to fp16 on ACT (Pool's Q7 cast measured 2.5x slower) ---
            x_nat = xnat_pool.tile([128, NSB * D], FP16)
            nc.scalar.copy(x_nat[:], x_raw[:])
            # --- transpose to xT [128 d_lo, (dblk, s)] ---
            xT = xt_pool.tile([128, ND * S], FP16)
            xT3 = xT[:].rearrange("p (db s) -> p db s", db=ND)
            for db in range(ND):
                ptr = ptr_pool.tile([128, S], FP16, tag="ptr")
                for sb_i in range(NSB):
                    nc.tensor.transpose(
                        ptr[:, sb_i * 128:(sb_i + 1) * 128],
                        x_nat[:, sb_i * D + db * 128:sb_i * D + (db + 1) * 128],
                        ident_h[:],
                    )
                # psum -> sbuf copy, all on DVE (fp16 2x mode; ACT is busy
                # with the f32->fp16 input cast)
                nc.vector.tensor_copy(xT3[:, db], ptr[:])
                nc.tensor.matmul(
                    pu[si * 32:si * 32 + NK, :],
                    w_sb[:, db * NK:(db + 1) * NK],
                    xT3[:, db],
                    start=(db == 0),
                    stop=(db == ND - 1),
                    tile_position=(0, si * 32),
                )
            if si == SI - 1:
                # --- stack of 4 batches complete: back to natural layout ---
                uhT = uhT_pool.tile([128, S], FP16, tag="uhT")
                nc.vector.tensor_copy(uhT[:], pu[:])
                pnat = ptr_pool.tile([128, NSB * 128], FP16, tag="ptr")
                for sh in range(NSB):
                    nc.tensor.transpose(
                        pnat[:, sh * 128:(sh + 1) * 128],
                        uhT[:, sh * 128:(sh + 1) * 128],
                        ident_h[:],
                    )
                # slice away the 32-alignment padding: (sh, si, 32) -> (sh, si, 25)
                nc.vector.tensor_copy(
                    uh_ap[:, st],
                    pnat[:]
                    .rearrange("p (sh si c) -> p sh si c", sh=NSB, si=SI)[:, :, :, 0:NK]
                    .rearrange("p sh si (n k) -> p sh si n k", n=NCAP),
                )

        # ---------------- routing for this group ----------------
        # Per iter i (cs = 1/5 on iter 0 via the lncs bias, else 1):
        #   c    = softmax_n(b)                   (skipped on i=0: c = 1/5)
        #   v    = sum_s c*u_hat                  (PE partition-sum, raw v)
        #   rnrm = cs * (cs^2*|v|^2 + eps)^-1/2   (ACT: exp(-.5*ln(.)+ln cs))
        #   outv = v * rnrm                       (= squashed output)
        #   b'   = sum_k outv*u_hat               (pvb broadcast + reduce)
        blog = None
        for it in range(ROUTINGS):
            cs = 1.0 / NCAP if it == 0 else 1.0
            if it == 0:
                t_ap = uh_ap
            else:
                expb = rt_pool.tile([128, G * NSB * NCAP], BF16, tag=f"expb{gp}")
                nc.scalar.activation(expb[:], blog[:], AF.Exp)
                den = rt_pool.tile([128, G * NSB], F32, tag=f"den{gp}")
                nc.vector.reduce_sum(
                    den[:],
                    expb[:].rearrange("p (bs n) -> p bs n", n=NCAP),
                    axis=AX.X,
                )
                rden = rt_pool.tile([128, G * NSB], F32, tag=f"rden{gp}")
                nc.vector.reciprocal(rden[:], den[:])
                c = rt_pool.tile([128, G * NSB * NCAP], FP16, tag=f"c{gp}")
                nc.vector.tensor_tensor(
                    c[:].rearrange(
                        "p (st sh si n) -> p st sh si n", st=GSI, sh=NSB, si=SI
                    ),
                    expb[:].rearrange(
                        "p (st sh si n) -> p st sh si n", st=GSI, sh=NSB, si=SI
                    ),
                    rden[:]
                    .rearrange("p (st sh si) -> p st sh si", st=GSI, sh=NSB)
                    .unsqueeze(4)
                    .broadcast_to((128, GSI, NSB, SI, NCAP)),
                    op=OP.mult,
                )
                c_b = (
                    c[:]
                    .rearrange(
                        "p (st sh si n) -> p st sh si n", st=GSI, sh=NSB, si=SI
                    )
                    .unsqueeze(5)
                    .broadcast_to((128, GSI, NSB, SI, NCAP, KDIM))
                )
                t = rt_pool.tile([128, G * NSB * NK], FP16, tag=f"t{gp}")
                t_ap = t[:].rearrange(
                    "p (st sh si n k) -> p st sh si n k",
                    st=GSI, sh=NSB, si=SI, n=NCAP,
                )
                nc.vector.tensor_tensor(t_ap, uh_ap, c_b, op=OP.mult)
            # ---- v[b,n,k] = sum_s t: partition sum via ones matmul ----
            pv = pv_pool.tile([1, G * NK], F32, tag=f"pv_{gp}")
            for sh in range(NSB):
                nc.tensor.matmul(
                    pv[:],
                    ones_col_h[:],
                    t_ap[:, :, sh],
                    start=(sh == 0),
                    stop=(sh == NSB - 1),
                )
            # ---- norm branch (all tiny): v_sb copy then Square, both on ACT
            v_sb = rt_pool.tile([1, G * NK], F32, tag=f"v_sb{gp}")
            nc.scalar.copy(v_sb[:], pv[:])
            sq = rt_pool.tile([1, G * NK], F32, tag=f"sq{gp}")
            nc.scalar.activation(sq[:], v_sb[:], AF.Square)
            s2 = rt_pool.tile([1, G * NCAP], F32, tag=f"s2{gp}")
            nc.vector.reduce_sum(
                s2[:], sq[:].rearrange("p (bn k) -> p bn k", k=KDIM), axis=AX.X
            )
            lns = rt_pool.tile([1, G * NCAP], F32, tag=f"lns{gp}")
            nc.scalar.activation(
                lns[:], s2[:], AF.Ln, bias=eps1[:], scale=cs * cs
            )
            rnrm = rt_pool.tile([1, G * NCAP], F32, tag=f"rnrm{gp}")
            if it == 0:
                nc.scalar.activation(rnrm[:], lns[:], AF.Exp, bias=lncs[:], scale=-0.5)
            else:
                nc.scalar.activation(rnrm[:], lns[:], AF.Exp, scale=-0.5)
            # outv = v * cs*rnrm  (squashed output; b-update = sum_k outv*uh)
            outv = rt_pool.tile([1, G * NK], F32, tag=f"outv{gp}")
            nc.vector.tensor_tensor(
                outv[:].rearrange("p (bn k) -> p bn k", k=KDIM),
                v_sb[:].rearrange("p (bn k) -> p bn k", k=KDIM),
                rnrm[:].unsqueeze(2).broadcast_to((1, G * NCAP, KDIM)),
                op=OP.mult,
            )
            if it < ROUTINGS - 1:
                pvb = pv_pool.tile([128, G * NK], F32, tag=f"pvb{gp}")
                nc.tensor.matmul(pvb[:], ones_row[:], outv[:], start=True, stop=True)
                tmp = rt_pool.tile([128, G * NSB * NK], FP16, tag=f"tmp{gp}")
                tmp_ap = tmp[:].rearrange(
                    "p (st sh si n k) -> p st sh si n k",
                    st=GSI, sh=NSB, si=SI, n=NCAP,
                )
                nc.vector.tensor_tensor(
                    tmp_ap,
                    uh_ap,
                    pvb[:]
                    .rearrange("p (st si n k) -> p st si n k", st=GSI, si=SI, n=NCAP)
                    .unsqueeze(2)
                    .broadcast_to((128, GSI, NSB, SI, NCAP, KDIM)),
                    op=OP.mult,
                )
                blog = rt_pool.tile([128, G * NSB * NCAP], FP16, tag=f"blog{gp}")
                with nc.allow_low_precision("5-term k-reduce of bounded logits"):
                    nc.vector.reduce_sum(
                        blog[:],
                        tmp[:].rearrange("p (bsn k) -> p bsn k", k=KDIM),
                        axis=AX.X,
                    )
            else:
                nc.sync.dma_start(
                    out[0:1, b_off * NK:(b_off + G) * NK],
                    outv[0:1, :],
                )
        b_off += G


def legalize_waits(nc):
    """This toolchain's walrus codegen accepts at most ONE sync wait per
    instruction ("Too many sync wait commands" otherwise) — and PE Matmult
    appears to take none safely. Hoist excess waits onto wait-only
    EventSemaphore instructions inserted just before, on the same engine
    (same pattern walrus already accepts for Tile's engine barriers)."""
    n = 0
    for fn in nc.m.functions:
        for blk in fn.blocks:
            new = []
            for inst in blk.instructions:
                si = inst.sync_info
                if si is not None and len(si.on_wait) > 0:
                    waits = list(si.on_wait)
                    keep = 0 if type(inst).__name__ == "InstMatmult" else 1
                    if len(waits) > keep:
                        for wt in waits[: len(waits) - keep]:
                            ev = mybir.InstEventSemaphore(
                                name=f"I-waitfix-{nc.next_id()}"
                            )
                            ev.engine = inst.engine
                            ev.sync_info = mybir.SyncInfo(on_wait=[wt], on_update=[])
                            new.append(ev)
                            n += 1
                        si.on_wait = waits[len(waits) - keep:]
                new.append(inst)
            blk.instructions = new
    return n


def build_caps_kernel(b_loc=16, groups=(8, 4, 4)):
    nc = bass.Bass(trn_type="TRN2", debug=False, target_bir_lowering=False)
    x = nc.dram_tensor("x", [b_loc * S, D], F32, kind="ExternalInput").ap()
    w = nc.dram_tensor("w", [D, NK], F32, kind="ExternalInput").ap()
    out = nc.dram_tensor("out", [1, b_loc * NK], F32, kind="ExternalOutput").ap()
    with tile.TileContext(nc) as tc:
        with ExitStack() as ctx:
            emit(ctx, tc, out, x, w, b_loc=b_loc, groups=groups)
    legalize_waits(nc)
    return nc


_KERNEL_CFG = dict(groups=(8, 4, 4))


def kernel(x: np.ndarray, W: np.ndarray) -> np.ndarray:
    from concourse.bass_utils import run_bass_kernel_spmd

    B, S_, D_ = x.shape
    assert (B, S_, D_) == (B_FULL, S, D)
    b_loc = B // N_CORES
    nc = build_caps_kernel(b_loc=b_loc, **_KERNEL_CFG)
    in_maps = [
        {
            "x": np.ascontiguousarray(
                x[i * b_loc:(i + 1) * b_loc].reshape(b_loc * S, D)
            ),
            "w": np.ascontiguousarray(W),
        }
        for i in range(N_CORES)
    ]
    res = run_bass_kernel_spmd(nc, in_maps, core_ids=list(range(N_CORES)))
    outs = [res.results[i]["out"].reshape(b_loc, NCAP, KDIM) for i in range(N_CORES)]
    return np.concatenate(outs, axis=0).astype(np.float32)


# revision 12
# speedup vs baseline: 1.3621x; 1.0338x over previous
"""Trainium2 Bass kernel for the capsule-routing layer (nn_Caps_Layer).

Computation (per batch b of x [B, S, D], W [D, 25]):
  u_hat = (x_b @ W).reshape(S, 5, 5)           # [S, n, k], col = n*5+k
  b0 = 0;  for 4 routing iters:
    c = softmax_n(b)                            # over the 5 capsules
    v[n,k] = sum_s c[n,s] u_hat[s,n,k]
    out = v / sqrt(sum_k v^2 + 1e-7)
    b[n,s] = sum_k out[n,k] u_hat[s,n,k]
Returns out [B, 5, 5].

Sharding: pure data-parallel over batch across 8 NeuronCores (16 batches
each); W replicated; no collectives.

v5 pipeline (per core):
  - x loaded f32 at full HWDGE rate (sync engine); cast f32->fp16 on ACT.
  - PE transposes at fp16 (1 cyc/row) into fp16 PSUM, two d-blocks per
    PSUM tile; psum->sbuf copies on DVE, one [128,1024] copy per pair.
  - main matmul fp16 with W columns PERMUTED to (k,n) order, so all
    downstream tensors are (.., k, n) with n innermost: broadcast-over-k
    operands keep a packed last dim -> DVE 2x mode on the big multiplies.
  - routing: the squash norm is folded into the broadcast (outv = v *
    cs*rnrm), rsqrt as exp(-0.5*ln(cs^2*s2+eps)) so ACT stays on ONE
    table set (copy/exp/ln all in natural_log_exp_and_others).
  - engine streams execute in order, so routing is emitted interleaved:
    ONE routing iteration of a finished group after each later batch's
    phase 1 (chains drain during phase 1); leftover chains at the end are
    emitted round-robin so they overlap each other.
"""

from contextlib import ExitStack

import math

import numpy as np

import concourse.bass as bass
import concourse.tile as tile
from concourse import mybir, masks

F32 = mybir.dt.float32
FP16 = mybir.dt.float16
BF16 = mybir.dt.bfloat16
AX = mybir.AxisListType
OP = mybir.AluOpType
AF = mybir.ActivationFunctionType

N_CORES = 8
B_FULL, S, D = 128, 512, 768
NCAP, KDIM = 5, 5
NK = NCAP * KDIM  # 25
ROUTINGS = 4
T_EPS = 1e-7

ND = D // 128   # 6 d-blocks
NSB = S // 128  # 4 s-blocks (= s_hi)
SI = 4          # batches stacked per [128,512] psum tile (offsets 0/32/64/96)


def emit(ctx, tc, out, x, w, b_loc=16, groups=(8, 4, 4)):
    """Emit the per-core kernel IR.

    out: [1, b_loc*25] f32; x: [b_loc*512, 768] f32; w: [768, 25] f32.
    """
    nc = tc.nc
    groups = list(groups)
    assert sum(groups) == b_loc
    assert all(g % SI == 0 for g in groups)

    const_pool = ctx.enter_context(tc.tile_pool(name="const", bufs=1))
    xraw_pool = ctx.enter_context(tc.tile_pool(name="xraw", bufs=3))
    xnat_pool = ctx.enter_context(tc.tile_pool(name="xnat", bufs=3))
    ptr_pool = ctx.enter_context(tc.tile_pool(name="ptr", bufs=2, space="PSUM"))
    xt_pool = ctx.enter_context(tc.tile_pool(name="xt", bufs=3))
    pu_pool = ctx.enter_context(tc.tile_pool(name="pu", bufs=2, space="PSUM"))
    uhT_pool = ctx.enter_context(tc.tile_pool(name="uhT", bufs=2))
    uh_pool = ctx.enter_context(tc.tile_pool(name="uh", bufs=2))
    rt_pool = ctx.enter_context(tc.tile_pool(name="rt", bufs=2))
    pv_pool = ctx.enter_context(tc.tile_pool(name="pv", bufs=1, space="PSUM"))

    # --- constants ---
    ident = const_pool.tile([128, 128], F32)
    masks.make_identity(nc, ident[:])
    ident_h = const_pool.tile([128, 128], FP16)
    nc.scalar.copy(ident_h[:], ident[:])
    w_raw = const_pool.tile([128, ND * NK], F32)
    # DRAM [768, 25] -> [128, (dblk, nk)]
    nc.sync.dma_start(
        w_raw[:].rearrange("p (nb k) -> p nb k", nb=ND),
        w.rearrange("(nb p) k -> p nb k", p=128),
    )
    # permute W's columns (n,k)->(k,n): everything downstream (pu
    # partitions, uhT, uh, pv, outv) is then (k,n)-ordered with n
    # innermost, which keeps broadcast-over-k APs packed for DVE 2x.
    w_sb = const_pool.tile([128, ND * NK], FP16)
    nc.scalar.copy(
        w_sb[:].rearrange("p (nb k n) -> p nb k n", k=KDIM, n=NCAP),
        w_raw[:].rearrange("p (nb n k) -> p nb k n", n=NCAP, k=KDIM),
    )

    ones_col_h = const_pool.tile([128, 1], FP16)
    nc.gpsimd.memset(ones_col_h[:], 1.0)
    ones_row = const_pool.tile([1, 128], F32)
    nc.gpsimd.memset(ones_row[:], 1.0)
    eps1 = const_pool.tile([1, 1], F32)
    nc.gpsimd.memset(eps1[:], T_EPS)
    # bias for folding cs=1/5 into the iter-0 rsqrt: exp(-.5*ln(..)+ln(cs))
    lncs = const_pool.tile([1, 1], F32)
    nc.gpsimd.memset(lncs[:], math.log(1.0 / NCAP))

    # HAM warm-up overlapping the first DMA (real matmuls at 2.4GHz after
    # ~3us of continuous PE activity; later the 6 pu-matmuls per batch and
    # the routing pv matmuls keep the clock gate awake).
    wps = pv_pool.tile([1, 128], F32, tag="pv_0")
    for _ in range(24):
        nc.tensor.matmul(wps[:], ones_col_h[:], ident_h[:], start=True, stop=True)

    # ---------------- routing (emitted one iteration at a time) ----------
    def routing_iter(rs):
        it = rs["it"]
        rs["it"] += 1
        G, gp, uh_ap, boff = rs["G"], rs["gp"], rs["uh_ap"], rs["b_off"]
        GSI = G // SI
        cs = 1.0 / NCAP if it == 0 else 1.0
        if it == 0:
            t_ap = uh_ap
        else:
            blog = rs["blog"]
            expb = rt_pool.tile([128, G * NSB * NCAP], BF16, tag=f"expb{gp}")
            nc.scalar.activation(expb[:], blog[:], AF.Exp)
            den = rt_pool.tile([128, G * NSB], F32, tag=f"den{gp}")
            nc.vector.reduce_sum(
                den[:],
                expb[:].rearrange("p (bs n) -> p bs n", n=NCAP),
                axis=AX.X,
            )
            rden = rt_pool.tile([128, G * NSB], F32, tag=f"rden{gp}")
            nc.vector.reciprocal(rden[:], den[:])
            c = rt_pool.tile([128, G * NSB * NCAP], FP16, tag=f"c{gp}")
            nc.vector.tensor_tensor(
                c[:].rearrange(
                    "p (st sh si n) -> p st sh si n", st=GSI, sh=NSB, si=SI
                ),
                expb[:].rearrange(
                    "p (st sh si n) -> p st sh si n", st=GSI, sh=NSB, si=SI
                ),
                rden[:]
                .rearrange("p (st sh si) -> p st sh si", st=GSI, sh=NSB)
                .unsqueeze(4)
                .broadcast_to((128, GSI, NSB, SI, NCAP)),
                op=OP.mult,
            )
            # broadcast c over k (dim 4); last dim n stays packed -> DVE 2x
            c_b = (
                c[:]
                .rearrange(
                    "p (st sh si n) -> p st sh si n", st=GSI, sh=NSB, si=SI
                )
                .unsqueeze(4)
                .broadcast_to((128, GSI, NSB, SI, KDIM, NCAP))
            )
            t = rt_pool.tile([128, G * NSB * NK], FP16, tag=f"t{gp}")
            t_ap = t[:].rearrange(
                "p (st sh si k n) -> p st sh si k n",
                st=GSI, sh=NSB, si=SI, k=KDIM,
            )
            nc.vector.tensor_tensor(t_ap, uh_ap, c_b, op=OP.mult)
        # ---- v[b,k,n] = sum_s t: partition sum via ones matmul ----
        pv = pv_pool.tile([1, G * NK], F32, tag=f"pv_{gp}")
        for sh in range(NSB):
            nc.tensor.matmul(
                pv[:],
                ones_col_h[:],
                t_ap[:, :, sh],
                start=(sh == 0),
                stop=(sh == NSB - 1),
            )
        # ---- norm branch (tiny ops): v_sb, sq, s2 on DVE; ln/exp on ACT
        v_sb = rt_pool.tile([1, G * NK], F32, tag=f"v_sb{gp}")
        nc.vector.tensor_copy(v_sb[:], pv[:])
        sq = rt_pool.tile([1, G * NK], F32, tag=f"sq{gp}")
        nc.vector.tensor_tensor(sq[:], v_sb[:], v_sb[:], op=OP.mult)
        s2 = rt_pool.tile([1, G * NCAP], F32, tag=f"s2{gp}")
        nc.vector.reduce_sum(
            s2[:],
            sq[:].rearrange("p (b k n) -> p b n k", k=KDIM, n=NCAP),
            axis=AX.X,
        )
        lns = rt_pool.tile([1, G * NCAP], F32, tag=f"lns{gp}")
        nc.scalar.activation(lns[:], s2[:], AF.Ln, bias=eps1[:], scale=cs * cs)
        rnrm = rt_pool.tile([1, G * NCAP], F32, tag=f"rnrm{gp}")
        if it == 0:
            nc.scalar.activation(rnrm[:], lns[:], AF.Exp, bias=lncs[:], scale=-0.5)
        else:
            nc.scalar.activation(rnrm[:], lns[:], AF.Exp, scale=-0.5)
        # outv[b,k,n] = v * cs*rnrm  (squashed output)
        outv = rt_pool.tile([1, G * NK], F32, tag=f"outv{gp}")
        nc.vector.tensor_tensor(
            outv[:].rearrange("p (b k n) -> p b k n", k=KDIM, n=NCAP),
            v_sb[:].rearrange("p (b k n) -> p b k n", k=KDIM, n=NCAP),
            rnrm[:]
            .rearrange("p (b n) -> p b n", n=NCAP)
            .unsqueeze(2)
            .broadcast_to((1, G, KDIM, NCAP)),
            op=OP.mult,
        )
        if it < ROUTINGS - 1:
            pvb = pv_pool.tile([128, G * NK], F32, tag=f"pvb{gp}")
            nc.tensor.matmul(pvb[:], ones_row[:], outv[:], start=True, stop=True)
            # stage pvb to SBUF fp16 (ACT) so tmp is all-fp16-SBUF -> DVE 2x
            pvh = rt_pool.tile([128, G * NK], FP16, tag=f"pvh{gp}")
            nc.scalar.copy(pvh[:], pvb[:])
            tmp = rt_pool.tile([128, G * NSB * NK], FP16, tag=f"tmp{gp}")
            tmp_ap = tmp[:].rearrange(
                "p (st sh si k n) -> p st sh si k n",
                st=GSI, sh=NSB, si=SI, k=KDIM,
            )
            nc.vector.tensor_tensor(
                tmp_ap,
                uh_ap,
                pvh[:]
                .rearrange("p (st si k n) -> p st si k n", st=GSI, si=SI, k=KDIM)
                .unsqueeze(2)
                .broadcast_to((128, GSI, NSB, SI, KDIM, NCAP)),
                op=OP.mult,
            )
            blog = rt_pool.tile([128, G * NSB * NCAP], FP16, tag=f"blog{gp}")
            with nc.allow_low_precision("5-term k-reduce of bounded logits"):
                nc.vector.reduce_sum(
                    blog[:],
                    tmp[:].rearrange(
                        "p (st sh si k n) -> p st sh si n k", st=GSI,
                        sh=NSB, si=SI, k=KDIM,
                    ),
                    axis=AX.X,
                )
            rs["blog"] = blog
        else:
            # permute (b,k,n) -> (b,n,k) for the DRAM layout, then DMA
            outp = rt_pool.tile([1, G * NK], F32, tag=f"outp{gp}")
            nc.vector.tensor_copy(
                outp[:].rearrange("p (b n k) -> p b n k", n=NCAP, k=KDIM),
                outv[:].rearrange("p (b k n) -> p b n k", k=KDIM, n=NCAP),
            )
            nc.sync.dma_start(
                out[0:1, boff * NK:(boff + G) * NK],
                outp[0:1, :],
            )

    route_q = []

    def emit_pending_iter():
        for rs in route_q:
            if rs["it"] < ROUTINGS:
                routing_iter(rs)
                return

    # ---------------- phase 1 + interleaved routing ----------------
    b_off = 0
    for g, G in enumerate(groups):
        gp = g % 2
        GSI = G // SI
        uh = uh_pool.tile([128, G * NSB * NK], FP16, tag="uh")
        # free layout: (st, sh, si, k, n) -- n innermost
        uh_ap = uh[:].rearrange(
            "p (st sh si k n) -> p st sh si k n", st=GSI, sh=NSB, si=SI, k=KDIM
        )
        pu = None
        for bi in range(G):
            b = b_off + bi
            st, si = bi // SI, bi % SI
            if si == 0:
                pu = pu_pool.tile([128, S], F32, tag="pu")
            # --- load x rows for batch b: [512, 768] -> f32 [128,(sb,d)] ---
            x_raw = xraw_pool.tile([128, NSB * D], F32)
            nc.sync.dma_start(
                x_raw[:].rearrange("p (sb d) -> p sb d", sb=NSB),
                x[b * S:(b + 1) * S, :].rearrange("(sb p) d -> p sb d", p=128),
            )
            # --- cast to fp16 on ACT ---
            x_nat = xnat_pool.tile([128, NSB * D], FP16)
            nc.scalar.copy(x_nat[:], x_raw[:])
            # --- transpose to xT [128 d_lo, (dblk, s)], 2 d-blocks per
            #     PSUM tile so the psum->sbuf copy amortizes ---
            xT = xt_pool.tile([128, ND * S], FP16)
            xT3 = xT[:].rearrange("p (db s) -> p db s", db=ND)
            for dp in range(ND // 2):
                ptr = ptr_pool.tile([128, 2 * S], FP16, tag="ptr")
                for dq in range(2):
                    db = dp * 2 + dq
                    for sb_i in range(NSB):
                        nc.tensor.transpose(
                            ptr[:, dq * S + sb_i * 128:dq * S + (sb_i + 1) * 128],
                            x_nat[:, sb_i * D + db * 128:sb_i * D + (db + 1) * 128],
                            ident_h[:],
                        )
                nc.vector.tensor_copy(
                    xT3[:, dp * 2:dp * 2 + 2].rearrange("p a s -> p (a s)"),
                    ptr[:],
                )
                for dq in range(2):
                    db = dp * 2 + dq
                    nc.tensor.matmul(
                        pu[si * 32:si * 32 + NK, :],
                        w_sb[:, db * NK:(db + 1) * NK],
                        xT3[:, db],
                        start=(db == 0),
                        stop=(db == ND - 1),
                        tile_position=(0, si * 32),
                    )
            if si == SI - 1:
                # --- stack of 4 batches complete: back to natural layout ---
                uhT = uhT_pool.tile([128, S], FP16, tag="uhT")
                nc.vector.tensor_copy(uhT[:], pu[:])
                pnat = ptr_pool.tile([128, NSB * 128], FP16, tag="ptr")
                for sh in range(NSB):
                    nc.tensor.transpose(
                        pnat[:, sh * 128:(sh + 1) * 128],
                        uhT[:, sh * 128:(sh + 1) * 128],
                        ident_h[:],
                    )
                # slice the 32-align padding: (sh, si, 32) -> (sh, si, 25)
                nc.vector.tensor_copy(
                    uh_ap[:, st].rearrange("p sh si k n -> p sh si (k n)"),
                    pnat[:]
                    .rearrange("p (sh si c) -> p sh si c", sh=NSB, si=SI)[
                        :, :, :, 0:NK
                    ],
                )
            # keep earlier groups' routing chains draining between batches
            emit_pending_iter()
        route_q.append(
            {"it": 0, "G": G, "gp": gp, "uh_ap": uh_ap, "b_off": b_off,
             "blog": None}
        )
        b_off += G

    # drain remaining chains round-robin so they overlap each other
    while any(rs["it"] < ROUTINGS for rs in route_q):
        for rs in route_q:
            if rs["it"] < ROUTINGS:
                routing_iter(rs)


def legalize_waits(nc):
    """This toolchain's walrus codegen accepts at most ONE sync wait per
    instruction ("Too many sync wait commands" otherwise) — and PE Matmult
    appears to take none safely. Hoist excess waits onto wait-only
    EventSemaphore instructions inserted just before, on the same engine
    (same pattern walrus already accepts for Tile's engine barriers)."""
    n = 0
    for fn in nc.m.functions:
        for blk in fn.blocks:
            new = []
            for inst in blk.instructions:
                si = inst.sync_info
                if si is not None and len(si.on_wait) > 0:
                    waits = list(si.on_wait)
                    keep = 0 if type(inst).__name__ == "InstMatmult" else 1
                    if len(waits) > keep:
                        for wt in waits[: len(waits) - keep]:
                            ev = mybir.InstEventSemaphore(
                                name=f"I-waitfix-{nc.next_id()}"
                            )
                            ev.engine = inst.engine
                            ev.sync_info = mybir.SyncInfo(on_wait=[wt], on_update=[])
                            new.append(ev)
                            n += 1
                        si.on_wait = waits[len(waits) - keep:]
                new.append(inst)
            blk.instructions = new
    return n


def build_caps_kernel(b_loc=16, groups=(8, 4, 4)):
    nc = bass.Bass(trn_type="TRN2", debug=False, target_bir_lowering=False)
    x = nc.dram_tensor("x", [b_loc * S, D], F32, kind="ExternalInput").ap()
    w = nc.dram_tensor("w", [D, NK], F32, kind="ExternalInput").ap()
    out = nc.dram_tensor("out", [1, b_loc * NK], F32, kind="ExternalOutput").ap()
    with tile.TileContext(nc) as tc:
        with ExitStack() as ctx:
            emit(ctx, tc, out, x, w, b_loc=b_loc, groups=groups)
    legalize_waits(nc)
    return nc


_KERNEL_CFG = dict(groups=(8, 4, 4))


def kernel(x: np.ndarray, W: np.ndarray) -> np.ndarray:
    from concourse.bass_utils import run_bass_kernel_spmd

    B, S_, D_ = x.shape
    assert (B, S_, D_) == (B_FULL, S, D)
    b_loc = B // N_CORES
    nc = build_caps_kernel(b_loc=b_loc, **_KERNEL_CFG)
    in_maps = [
        {
            "x": np.ascontiguousarray(
                x[i * b_loc:(i + 1) * b_loc].reshape(b_loc * S, D)
            ),
            "w": np.ascontiguousarray(W),
        }
        for i in range(N_CORES)
    ]
    res = run_bass_kernel_spmd(nc, in_maps, core_ids=list(range(N_CORES)))
    outs = [res.results[i]["out"].reshape(b_loc, NCAP, KDIM) for i in range(N_CORES)]
    return np.concatenate(outs, axis=0).astype(np.float32)


# revision 15
# speedup vs baseline: 1.3849x; 1.0167x over previous
"""Trainium2 Bass kernel for the capsule-routing layer (nn_Caps_Layer).

Computation (per batch b of x [B, S, D], W [D, 25]):
  u_hat = (x_b @ W).reshape(S, 5, 5)           # [S, n, k], col = n*5+k
  b0 = 0;  for 4 routing iters:
    c = softmax_n(b)                            # over the 5 capsules
    v[n,k] = sum_s c[n,s] u_hat[s,n,k]
    out = v / sqrt(sum_k v^2 + 1e-7)
    b[n,s] = sum_k out[n,k] u_hat[s,n,k]
Returns out [B, 5, 5].

Sharding: pure data-parallel over batch across 8 NeuronCores (16 batches
each); W replicated; no collectives.

v5 pipeline (per core):
  - x loaded via gpsimd SWDGE casting DMA (f32 DRAM -> fp16 SBUF), which
    sustains the same ~330 GB/s as plain HWDGE loads.
  - PE transposes at fp16 (1 cyc/row) into fp16 PSUM, two d-blocks per
    PSUM tile; psum->sbuf copies on DVE, one [128,1024] copy per pair.
  - main matmul fp16 with W columns PERMUTED to (k,n) order, so all
    downstream tensors are (.., k, n) with n innermost: broadcast-over-k
    operands keep a packed last dim -> DVE 2x mode on the big multiplies.
  - routing: the squash norm is folded into the broadcast (outv = v *
    cs*rnrm), rsqrt as exp(-0.5*ln(cs^2*s2+eps)) so ACT stays on ONE
    table set (copy/exp/ln all in natural_log_exp_and_others).
  - engine streams execute in order, so routing is emitted interleaved:
    ONE routing iteration of a finished group after each later batch's
    phase 1 (chains drain during phase 1); leftover chains at the end are
    emitted round-robin so they overlap each other.
"""

from contextlib import ExitStack

import math

import numpy as np

import concourse.bass as bass
import concourse.tile as tile
from concourse import mybir, masks

F32 = mybir.dt.float32
FP16 = mybir.dt.float16
BF16 = mybir.dt.bfloat16
AX = mybir.AxisListType
OP = mybir.AluOpType
AF = mybir.ActivationFunctionType

N_CORES = 8
B_FULL, S, D = 128, 512, 768
NCAP, KDIM = 5, 5
NK = NCAP * KDIM  # 25
ROUTINGS = 4
T_EPS = 1e-7

ND = D // 128   # 6 d-blocks
NSB = S // 128  # 4 s-blocks (= s_hi)
SI = 4          # batches stacked per [128,512] psum tile (offsets 0/32/64/96)


def emit(ctx, tc, out, x, w, b_loc=16, groups=(8, 4, 4)):
    """Emit the per-core kernel IR.

    out: [1, b_loc*25] f32; x: [b_loc*512, 768] f32; w: [768, 25] f32.
    """
    nc = tc.nc
    groups = list(groups)
    assert sum(groups) == b_loc
    assert all(g % SI == 0 for g in groups)

    const_pool = ctx.enter_context(tc.tile_pool(name="const", bufs=1))
    xnat_pool = ctx.enter_context(tc.tile_pool(name="xnat", bufs=4))
    ptr_pool = ctx.enter_context(tc.tile_pool(name="ptr", bufs=2, space="PSUM"))
    xt_pool = ctx.enter_context(tc.tile_pool(name="xt", bufs=3))
    pu_pool = ctx.enter_context(tc.tile_pool(name="pu", bufs=2, space="PSUM"))
    uhT_pool = ctx.enter_context(tc.tile_pool(name="uhT", bufs=2))
    uh_pool = ctx.enter_context(tc.tile_pool(name="uh", bufs=2))
    rt_pool = ctx.enter_context(tc.tile_pool(name="rt", bufs=2))
    pv_pool = ctx.enter_context(tc.tile_pool(name="pv", bufs=1, space="PSUM"))

    # --- constants ---
    ident = const_pool.tile([128, 128], F32)
    masks.make_identity(nc, ident[:])
    ident_h = const_pool.tile([128, 128], FP16)
    nc.scalar.copy(ident_h[:], ident[:])
    w_raw = const_pool.tile([128, ND * NK], F32)
    # DRAM [768, 25] -> [128, (dblk, nk)]
    nc.sync.dma_start(
        w_raw[:].rearrange("p (nb k) -> p nb k", nb=ND),
        w.rearrange("(nb p) k -> p nb k", p=128),
    )
    # permute W's columns (n,k)->(k,n): everything downstream (pu
    # partitions, uhT, uh, pv, outv) is then (k,n)-ordered with n
    # innermost, which keeps broadcast-over-k APs packed for DVE 2x.
    w_sb = const_pool.tile([128, ND * NK], FP16)
    nc.scalar.copy(
        w_sb[:].rearrange("p (nb k n) -> p nb k n", k=KDIM, n=NCAP),
        w_raw[:].rearrange("p (nb n k) -> p nb k n", n=NCAP, k=KDIM),
    )

    ones_col_h = const_pool.tile([128, 1], FP16)
    nc.gpsimd.memset(ones_col_h[:], 1.0)
    ones_row = const_pool.tile([1, 128], F32)
    nc.gpsimd.memset(ones_row[:], 1.0)
    eps1 = const_pool.tile([1, 1], F32)
    nc.gpsimd.memset(eps1[:], T_EPS)
    # bias for folding cs=1/5 into the iter-0 rsqrt: exp(-.5*ln(..)+ln(cs))
    lncs = const_pool.tile([1, 1], F32)
    nc.gpsimd.memset(lncs[:], math.log(1.0 / NCAP))

    # HAM warm-up overlapping the first DMA (real matmuls at 2.4GHz after
    # ~3us of continuous PE activity; later the 6 pu-matmuls per batch and
    # the routing pv matmuls keep the clock gate awake).
    wps = pv_pool.tile([1, 128], F32, tag="pv_0")
    for _ in range(24):
        nc.tensor.matmul(wps[:], ones_col_h[:], ident_h[:], start=True, stop=True)

    # ---------------- routing (emitted one iteration at a time) ----------
    def routing_iter(rs):
        it = rs["it"]
        rs["it"] += 1
        G, gp, uh_ap, boff = rs["G"], rs["gp"], rs["uh_ap"], rs["b_off"]
        GSI = G // SI
        cs = 1.0 / NCAP if it == 0 else 1.0
        if it == 0:
            t_ap = uh_ap
        else:
            blog = rs["blog"]
            expb = rt_pool.tile([128, G * NSB * NCAP], BF16, tag=f"expb{gp}")
            nc.scalar.activation(expb[:], blog[:], AF.Exp)
            den = rt_pool.tile([128, G * NSB], F32, tag=f"den{gp}")
            nc.vector.reduce_sum(
                den[:],
                expb[:].rearrange("p (bs n) -> p bs n", n=NCAP),
                axis=AX.X,
            )
            rden = rt_pool.tile([128, G * NSB], F32, tag=f"rden{gp}")
            nc.vector.reciprocal(rden[:], den[:])
            c = rt_pool.tile([128, G * NSB * NCAP], FP16, tag=f"c{gp}")
            nc.vector.tensor_tensor(
                c[:].rearrange(
                    "p (st sh si n) -> p st sh si n", st=GSI, sh=NSB, si=SI
                ),
                expb[:].rearrange(
                    "p (st sh si n) -> p st sh si n", st=GSI, sh=NSB, si=SI
                ),
                rden[:]
                .rearrange("p (st sh si) -> p st sh si", st=GSI, sh=NSB)
                .unsqueeze(4)
                .broadcast_to((128, GSI, NSB, SI, NCAP)),
                op=OP.mult,
            )
            # broadcast c over k (dim 4); last dim n stays packed -> DVE 2x
            c_b = (
                c[:]
                .rearrange(
                    "p (st sh si n) -> p st sh si n", st=GSI, sh=NSB, si=SI
                )
                .unsqueeze(4)
                .broadcast_to((128, GSI, NSB, SI, KDIM, NCAP))
            )
            t = rt_pool.tile([128, G * NSB * NK], FP16, tag=f"t{gp}")
            t_ap = t[:].rearrange(
                "p (st sh si k n) -> p st sh si k n",
                st=GSI, sh=NSB, si=SI, k=KDIM,
            )
            nc.vector.tensor_tensor(t_ap, uh_ap, c_b, op=OP.mult)
        # ---- v[b,k,n] = sum_s t: partition sum via ones matmul ----
        pv = pv_pool.tile([1, G * NK], F32, tag=f"pv_{gp}")
        for sh in range(NSB):
            nc.tensor.matmul(
                pv[:],
                ones_col_h[:],
                t_ap[:, :, sh],
                start=(sh == 0),
                stop=(sh == NSB - 1),
            )
        # ---- norm branch (tiny ops): v_sb, sq, s2 on DVE; ln/exp on ACT
        v_sb = rt_pool.tile([1, G * NK], F32, tag=f"v_sb{gp}")
        nc.vector.tensor_copy(v_sb[:], pv[:])
        sq = rt_pool.tile([1, G * NK], F32, tag=f"sq{gp}")
        nc.vector.tensor_tensor(sq[:], v_sb[:], v_sb[:], op=OP.mult)
        s2 = rt_pool.tile([1, G * NCAP], F32, tag=f"s2{gp}")
        nc.vector.reduce_sum(
            s2[:],
            sq[:].rearrange("p (b k n) -> p b n k", k=KDIM, n=NCAP),
            axis=AX.X,
        )
        lns = rt_pool.tile([1, G * NCAP], F32, tag=f"lns{gp}")
        nc.scalar.activation(lns[:], s2[:], AF.Ln, bias=eps1[:], scale=cs * cs)
        rnrm = rt_pool.tile([1, G * NCAP], F32, tag=f"rnrm{gp}")
        if it == 0:
            nc.scalar.activation(rnrm[:], lns[:], AF.Exp, bias=lncs[:], scale=-0.5)
        else:
            nc.scalar.activation(rnrm[:], lns[:], AF.Exp, scale=-0.5)
        # outv[b,k,n] = v * cs*rnrm  (squashed output)
        outv = rt_pool.tile([1, G * NK], F32, tag=f"outv{gp}")
        nc.vector.tensor_tensor(
            outv[:].rearrange("p (b k n) -> p b k n", k=KDIM, n=NCAP),
            v_sb[:].rearrange("p (b k n) -> p b k n", k=KDIM, n=NCAP),
            rnrm[:]
            .rearrange("p (b n) -> p b n", n=NCAP)
            .unsqueeze(2)
            .broadcast_to((1, G, KDIM, NCAP)),
            op=OP.mult,
        )
        if it < ROUTINGS - 1:
            pvb = pv_pool.tile([128, G * NK], F32, tag=f"pvb{gp}")
            nc.tensor.matmul(pvb[:], ones_row[:], outv[:], start=True, stop=True)
            # stage pvb to SBUF fp16 (ACT) so tmp is all-fp16-SBUF -> DVE 2x
            pvh = rt_pool.tile([128, G * NK], FP16, tag=f"pvh{gp}")
            nc.scalar.copy(pvh[:], pvb[:])
            tmp = rt_pool.tile([128, G * NSB * NK], FP16, tag=f"tmp{gp}")
            tmp_ap = tmp[:].rearrange(
                "p (st sh si k n) -> p st sh si k n",
                st=GSI, sh=NSB, si=SI, k=KDIM,
            )
            nc.vector.tensor_tensor(
                tmp_ap,
                uh_ap,
                pvh[:]
                .rearrange("p (st si k n) -> p st si k n", st=GSI, si=SI, k=KDIM)
                .unsqueeze(2)
                .broadcast_to((128, GSI, NSB, SI, KDIM, NCAP)),
                op=OP.mult,
            )
            blog = rt_pool.tile([128, G * NSB * NCAP], FP16, tag=f"blog{gp}")
            with nc.allow_low_precision("5-term k-reduce of bounded logits"):
                nc.vector.reduce_sum(
                    blog[:],
                    tmp[:].rearrange(
                        "p (st sh si k n) -> p st sh si n k", st=GSI,
                        sh=NSB, si=SI, k=KDIM,
                    ),
                    axis=AX.X,
                )
            rs["blog"] = blog
        else:
            # permute (b,k,n) -> (b,n,k) for the DRAM layout, then DMA
            outp = rt_pool.tile([1, G * NK], F32, tag=f"outp{gp}")
            nc.vector.tensor_copy(
                outp[:].rearrange("p (b n k) -> p b n k", n=NCAP, k=KDIM),
                outv[:].rearrange("p (b k n) -> p b n k", k=KDIM, n=NCAP),
            )
            nc.sync.dma_start(
                out[0:1, boff * NK:(boff + G) * NK],
                outp[0:1, :],
            )

    route_q = []

    def emit_pending_iter():
        for rs in route_q:
            if rs["it"] < ROUTINGS:
                routing_iter(rs)
                return

    # ---------------- phase 1 + interleaved routing ----------------
    b_off = 0
    for g, G in enumerate(groups):
        gp = g % 2
        GSI = G // SI
        uh = uh_pool.tile([128, G * NSB * NK], FP16, tag="uh")
        # free layout: (st, sh, si, k, n) -- n innermost
        uh_ap = uh[:].rearrange(
            "p (st sh si k n) -> p st sh si k n", st=GSI, sh=NSB, si=SI, k=KDIM
        )
        pu = None
        for bi in range(G):
            b = b_off + bi
            st, si = bi // SI, bi % SI
            if si == 0:
                pu = pu_pool.tile([128, S], F32, tag="pu")
            # --- load x rows for batch b: SWDGE casting DMA f32 -> fp16
            #     (measured same ~330 GB/s as HWDGE; conversion is free) ---
            x_nat = xnat_pool.tile([128, NSB * D], FP16)
            nc.gpsimd.dma_start(
                x_nat[:].rearrange("p (sb d) -> p sb d", sb=NSB),
                x[b * S:(b + 1) * S, :].rearrange("(sb p) d -> p sb d", p=128),
            )
            # --- transpose to xT [128 d_lo, (dblk, s)], 2 d-blocks per
            #     PSUM tile so the psum->sbuf copy amortizes ---
            xT = xt_pool.tile([128, ND * S], FP16)
            xT3 = xT[:].rearrange("p (db s) -> p db s", db=ND)
            for dp in range(ND // 2):
                ptr = ptr_pool.tile([128, 2 * S], FP16, tag="ptr")
                for dq in range(2):
                    db = dp * 2 + dq
                    for sb_i in range(NSB):
                        nc.tensor.transpose(
                            ptr[:, dq * S + sb_i * 128:dq * S + (sb_i + 1) * 128],
                            x_nat[:, sb_i * D + db * 128:sb_i * D + (db + 1) * 128],
                            ident_h[:],
                        )
                nc.vector.tensor_copy(
                    xT3[:, dp * 2:dp * 2 + 2].rearrange("p a s -> p (a s)"),
                    ptr[:],
                )
                for dq in range(2):
                    db = dp * 2 + dq
                    nc.tensor.matmul(
                        pu[si * 32:si * 32 + NK, :],
                        w_sb[:, db * NK:(db + 1) * NK],
                        xT3[:, db],
                        start=(db == 0),
                        stop=(db == ND - 1),
                        tile_position=(0, si * 32),
                    )
            if si == SI - 1:
                # --- stack of 4 batches complete: back to natural layout ---
                uhT = uhT_pool.tile([128, S], FP16, tag="uhT")
                nc.vector.tensor_copy(uhT[:], pu[:])
                pnat = ptr_pool.tile([128, NSB * 128], FP16, tag="ptr")
                for sh in range(NSB):
                    nc.tensor.transpose(
                        pnat[:, sh * 128:(sh + 1) * 128],
                        uhT[:, sh * 128:(sh + 1) * 128],
                        ident_h[:],
                    )
                # slice the 32-align padding: (sh, si, 32) -> (sh, si, 25)
                nc.vector.tensor_copy(
                    uh_ap[:, st].rearrange("p sh si k n -> p sh si (k n)"),
                    pnat[:]
                    .rearrange("p (sh si c) -> p sh si c", sh=NSB, si=SI)[
                        :, :, :, 0:NK
                    ],
                )
            # keep earlier groups' routing chains draining between batches
            emit_pending_iter()
        route_q.append(
            {"it": 0, "G": G, "gp": gp, "uh_ap": uh_ap, "b_off": b_off,
             "blog": None}
        )
        b_off += G

    # drain remaining chains round-robin so they overlap each other
    while any(rs["it"] < ROUTINGS for rs in route_q):
        for rs in route_q:
            if rs["it"] < ROUTINGS:
                routing_iter(rs)


def legalize_waits(nc):
    """This toolchain's walrus codegen accepts at most ONE sync wait per
    instruction ("Too many sync wait commands" otherwise) — and PE Matmult
    appears to take none safely. Hoist excess waits onto wait-only
    EventSemaphore instructions inserted just before, on the same engine
    (same pattern walrus already accepts for Tile's engine barriers)."""
    n = 0
    for fn in nc.m.functions:
        for blk in fn.blocks:
            new = []
            for inst in blk.instructions:
                si = inst.sync_info
                if si is not None and len(si.on_wait) > 0:
                    waits = list(si.on_wait)
                    keep = 0 if type(inst).__name__ == "InstMatmult" else 1
                    if len(waits) > keep:
                        for wt in waits[: len(waits) - keep]:
                            ev = mybir.InstEventSemaphore(
                                name=f"I-waitfix-{nc.next_id()}"
                            )
                            ev.engine = inst.engine
                            ev.sync_info = mybir.SyncInfo(on_wait=[wt], on_update=[])
                            new.append(ev)
                            n += 1
                        si.on_wait = waits[len(waits) - keep:]
                new.append(inst)
            blk.instructions = new
    return n


def build_caps_kernel(b_loc=16, groups=(8, 4, 4)):
    nc = bass.Bass(trn_type="TRN2", debug=False, target_bir_lowering=False)
    x = nc.dram_tensor("x", [b_loc * S, D], F32, kind="ExternalInput").ap()
    w = nc.dram_tensor("w", [D, NK], F32, kind="ExternalInput").ap()
    out = nc.dram_tensor("out", [1, b_loc * NK], F32, kind="ExternalOutput").ap()
    with tile.TileContext(nc) as tc:
        with ExitStack() as ctx:
            emit(ctx, tc, out, x, w, b_loc=b_loc, groups=groups)
    legalize_waits(nc)
    return nc


_KERNEL_CFG = dict(groups=(8, 4, 4))


def kernel(x: np.ndarray, W: np.ndarray) -> np.ndarray:
    from concourse.bass_utils import run_bass_kernel_spmd

    B, S_, D_ = x.shape
    assert (B, S_, D_) == (B_FULL, S, D)
    b_loc = B // N_CORES
    nc = build_caps_kernel(b_loc=b_loc, **_KERNEL_CFG)
    in_maps = [
        {
            "x": np.ascontiguousarray(
                x[i * b_loc:(i + 1) * b_loc].reshape(b_loc * S, D)
            ),
            "w": np.ascontiguousarray(W),
        }
        for i in range(N_CORES)
    ]
    res = run_bass_kernel_spmd(nc, in_maps, core_ids=list(range(N_CORES)))
    outs = [res.results[i]["out"].reshape(b_loc, NCAP, KDIM) for i in range(N_CORES)]
    return np.concatenate(outs, axis=0).astype(np.float32)
